# revision 75
# baseline (speedup 1.0000x reference)
"""Multi-head attention (B=4, S=2048, H=8 heads, d_head=16) on 8 trn2 cores.

Sharding: one head per core. Per head/batch, masked-softmax attention with a
transposed-scores dataflow and a two-engine exp pipeline:

    S^T[k, q] = matmul(lhsT=K_dT[49,2,128], rhs=Q_dT[49,2,512])  fp8 e4m3
        DoubleRow (0.5 cycles/row): Q and K split hi/mid/lo in fp8 on the
        host with scale-balanced pairings (6 cross terms, ~0.003 logit
        err), plus two shift rows (3.25 + dlo) * valid(k) that pre-bias
        the logits so both exp paths below need no additive constant.
        Invalid key columns are zeroed on the host (replaces the -1e30
        mask: their weights underflow to 0).

    P^T = exp(4*S^T - 75)  computed on TWO engines, alternating per unit
        (unit = one k-tile x 1024 q columns; greedy weighted assignment,
        last unit forced to ScalarE to shorten the drain critical path):
        - ScalarE: activation Exp (exact), bias = -75 - 4*(3.25+dlo)
        - DVE: Schraudolph bit-trick exp: int16 bits = rint(max(st*A, 0))
          with A = 4*128/ln2, written through an int16 bitcast view of the
          bf16 P^T tile. Round-to-nearest + saturation verified on HW.
          ~+-3% relative error on ~half the weights; rel-err budget 2e-2.

    out^T[q, j] += matmul(lhsT=P^T[128,128q], rhs=VO[128,17])  per q-tile:
        transposed AV: P^T is the stationary operand, the moving operand is
        the tiny [128 keys, 17] V|ones block, so each AV matmul streams only
        17 rows. Column 16 (ones*valid) accumulates the softmax denominator.
        Accumulated over k-tiles in PSUM; host divides num/den.

Weight loads are free in the cost model; matmul cost = moving free size.
Engine-busy per unit: ScalarE ~1038ns, DVE ~1192ns, PE ~270ns. Greedy
weighted assignment keeps both exp engines saturated; the kernel is
exp-throughput-bound at ~0.58us/unit (76 units). AV matmuls are emitted
lag-6 behind S^T so the in-order PE never blocks st production on an
in-flight exp; PSUM: 3 st bufs (6 banks) + 2 ot accumulators (2 banks).
A later start=True matmul in a PSUM bank discards other regions'
uncommitted accumulation (verified on HW), so each batch's ot bank is
opened by one zeroing start=True matmul and all AV matmuls accumulate
with start=False.

DMA: the transfer server is one serialized FIFO (by generation-end +
DGE delay) and HWDGE generation is a single engine shared by the
sync/scalar rings, so the critical first-unit set is kept to TWO
generations running in parallel (a strided 3D-AP DMA fetches both
DoubleRow r-slices of the q half in one go on sync HWDGE; kT b0 rides
first-in-queue on the Pool SWDGE path) while everything else queues
behind. Output copies split at the q-half boundary: under half-major
unit order a batch's first 8 regions finish many units before its
last, so their PSUM->SBUF copy + DMA hide entirely in steady state.
"""

import ml_dtypes
import numpy as np

import concourse.bass as bass
import concourse.tile as tile
from concourse import bacc, mybir
from concourse.ap import AP as BassAP
from concourse.bass_utils import run_bass_kernel_spmd


def _ap3(ap, d1, d1_stride):
    """Expand a 2D AP [P, N] to [P, d1, N] with the middle dim strided —
    the [partition, 2, moving] layout DoubleRow matmuls expect."""
    (pstride, pcount), (fstride, fcount) = list(ap.ap)
    assert fstride == 1
    return BassAP(
        ap.tensor, ap.offset,
        [[pstride, pcount], [d1_stride, d1], [1, fcount]],
    )

B = 4
S = 2048
H = 8
DH = 16
KT_TILE = 128
VO_W = 17  # 16 v dims + ones column

bf16 = ml_dtypes.bfloat16
f8 = ml_dtypes.float8_e4m3

DHI = 3.25
# shift-lo row is stored in fp8 as f8(64*dlo) paired with q-row 2^-6
DLO = float(np.float32(np.asarray(-7.019043e-04 * 64.0, dtype=np.float32).astype(f8)) / 64.0)
SH_A = 738.65986  # 4 * 128 / ln(2), f32
BIAS = -75.0 - 4.0 * (DHI + DLO)  # ScalarE activation bias

F32 = mybir.dt.float32
BF16 = mybir.dt.bfloat16
I16 = mybir.dt.int16
F8E4 = mybir.dt.float8e4

# per-unit engine busy estimates (ns) for the greedy assignment
import os as _os
SC_UNIT = 1024 * 0.8333 + 185
DV_ADJ = float(_os.environ.get("ATTN_DADJ", "40"))
DV_UNIT = 1024 * 1.0417 + 125
SC_COPY = 272 * 0.8333 + 185
DV_COPY = 272 * 1.0417 + 125

_cache = {}


def _build(nbs):
    nb_total = sum(nbs)

    nc = bacc.Bacc(
        "TRN2",
        target_bir_lowering=False,
        debug=False,
        num_devices=8,
    )

    # fp8 DoubleRow layouts: 98 contraction rows as [49 partitions, 2]
    # qT: [49, r*2048 + q]; kT: [49, t*256 + r*128 + m]
    qT_d = nc.dram_tensor(
        "qT", [B, 49, 2 * S], F8E4, kind="ExternalInput"
    ).ap()
    kT_d = nc.dram_tensor(
        "kT", [49, nb_total * 256], F8E4, kind="ExternalInput"
    ).ap()
    vo_d = nc.dram_tensor(
        "vo", [128, nb_total * VO_W], BF16, kind="ExternalInput"
    ).ap()
    out_d = nc.dram_tensor(
        "outT", [B, 128, 16 * VO_W], F32, kind="ExternalOutput"
    ).ap()

    with tile.TileContext(nc) as tc:
        with (
            tc.tile_pool(name="const", bufs=1) as const,
            tc.tile_pool(name="pt", bufs=9) as ptpool,
            tc.tile_pool(name="st", bufs=3, space="PSUM") as stpool,
            tc.tile_pool(name="ot", bufs=2, space="PSUM") as otpool,
            tc.tile_pool(name="ob", bufs=2) as obpool,
        ):
            q_tiles = [
                const.tile([49, 2 * S], F8E4, tag=f"qT{b}", name=f"qT{b}")
                for b in range(B)
            ]
            kT_t = const.tile([49, nb_total * 256], F8E4, tag="kT")
            vo_t = const.tile([128, nb_total * VO_W], BF16, tag="vo")

            # Critical-path DMAs first: kT tile 0 + vo on the sync HWDGE
            # ring, qT batch 0 in 512-col chunks on the scalar ring so the
            # first S^T matmul's input lands as early as possible. Bulk qT
            # for later batches rides the sync ring; bulk kT via gpsimd
            # SWDGE (Pool engine is otherwise idle).
            # DMA plan. The DMA transfer server is a single serialized
            # resource (FIFO by eligibility), and HWDGE generation is one
            # engine shared by the sync/scalar rings (~630ns per DMA) while
            # SWDGE generation runs separately on the Pool engine. Critical
            # order: qc0, qc1, kT0 feed the first unit; qc2+qc3 (second
            # half of qT b0) go via SWDGE so their generation overlaps;
            # bulk rides the sync ring BEHIND the critical set so its
            # transfers queue after them.
            pewarm = const.tile([98, 512], BF16, tag="pewarm")
            nc.vector.memset(pewarm[:], 0.0)
            # critical (HWDGE, 3 generations): both r-slices of qT b0's
            # first q-half on the scalar ring + all of kT b0 on sync.
            # The h1 q-slices and vo ride SWDGE (Pool) so they don't take
            # HWDGE slots ahead of the critical set.
            # one strided DMA brings BOTH r-slices of a q half: critical
            # set is just two generations (q-h0 on sync HWDGE, kT b0 first
            # on SWDGE), running in parallel
            nc.gpsimd.dma_start(
                kT_t[:, 0:nbs[0] * 256], kT_d[:, 0:nbs[0] * 256]
            )
            nc.sync.dma_start(
                _ap3(q_tiles[0][:, 0:1024], 2, 2048),
                _ap3(qT_d[0][:, 0:1024], 2, 2048),
            )
            nc.gpsimd.dma_start(
                _ap3(q_tiles[0][:, 1024:2048], 2, 2048),
                _ap3(qT_d[0][:, 1024:2048], 2, 2048),
            )
            nc.sync.dma_start(vo_t[:], vo_d)
            # Bias column for the ScalarE Exp activation.
            bias_t = const.tile([128, 1], F32, tag="bias")
            nc.vector.memset(bias_t[:], BIAS)
            # Zero moving-operand for the per-batch PSUM-bank-opening
            # matmul (see emit_av: one start=True matmul zeroes the ot
            # region; all real AV matmuls then accumulate with
            # start=False, since a later start=True in the same bank
            # discards other regions' uncommitted accumulation).
            zvo_t = const.tile([128, 16 * VO_W], BF16, tag="zvo")
            nc.vector.memset(zvo_t[:], 0.0)
            # Prefetch the exp table set on ScalarE while input DMAs run.
            warm = const.tile([1, 1], F32, tag="warm")
            nc.vector.memset(warm[:], 0.0)
            nc.scalar.activation(
                warm[:], warm[:], mybir.ActivationFunctionType.Exp
            )
            # Two dummy matmuls start the PE p-state ramp clock early; the
            # ramp counts wall time from the first PE instruction, so the
            # steady-state matmuls all run at full clock.
            st_w = stpool.tile([128, 1024], F32, tag="st")
            for j in range(2):
                nc.tensor.matmul(
                    st_w[:, 512 * j:512 * (j + 1)],
                    pewarm[:, 0:128],
                    pewarm[:],
                    start=True,
                    stop=True,
                )
            for b in range(1, B):
                off = sum(nbs[:b])
                nb = nbs[b]
                ring = nc.sync if b == 1 else nc.gpsimd
                ring.dma_start(q_tiles[b][:], qT_d[b])
                ring.dma_start(
                    kT_t[:, off * 256:(off + nb) * 256],
                    kT_d[:, off * 256:(off + nb) * 256],
                )

            # Unit list: one unit = one k-tile x 1024 q columns. Batch 0 is
            # half-major (all half-0 units first) so the first units only
            # need the first half of qT batch 0 — its 512-col DMA chunks
            # arrive serially on the scalar ring.
            units = []
            for b in range(B):
                if b == 0:
                    for half in range(2):
                        for kt in range(nbs[0]):
                            units.append(
                                (b, kt, kt, half, kt == 0, kt == nbs[0] - 1)
                            )
                else:
                    for half in range(2):
                        for kt in range(nbs[b]):
                            t = sum(nbs[:b]) + kt
                            units.append(
                                (b, kt, t, half, kt == 0, kt == nbs[b] - 1)
                            )

            # Greedy weighted engine assignment for the exp units (+ the
            # batch-end PSUM->SBUF copies).
            eng = []
            busy = {"S": 0.0, "D": 0.0}
            copy_eng = {}
            for u, (b, kt, t, half, first, last) in enumerate(units):
                if busy["S"] + SC_UNIT <= busy["D"] + DV_UNIT + DV_ADJ:
                    eng.append("S")
                    busy["S"] += SC_UNIT
                else:
                    eng.append("D")
                    busy["D"] += DV_UNIT
                if last and half == 1:
                    if busy["S"] + SC_COPY <= busy["D"] + DV_COPY:
                        copy_eng[b] = "S"
                        busy["S"] += SC_COPY
                    else:
                        copy_eng[b] = "D"
                        busy["D"] += DV_COPY

            sts = {}
            pts = {}
            ots = {}
            obs = {}

            def emit_st(u):
                b, kt, t, half, first, last = units[u]
                st = stpool.tile([128, 1024], F32, tag="st")
                for j in range(2):
                    qs = 1024 * half + 512 * j
                    nc.tensor.matmul(
                        st[:, 512 * j:512 * (j + 1)],
                        _ap3(kT_t[:, t * 256:(t + 1) * 256][:, 0:128], 2, 128),
                        _ap3(q_tiles[b][:, qs:qs + 512], 2, 2048),
                        start=True,
                        stop=True,
                        perf_mode=mybir.MatmulPerfMode.DoubleRow,
                    )
                pt = ptpool.tile([128, 1024], BF16, tag="pt")
                if u == len(units) - 1 and _os.environ.get("ATTN_LSPL"):
                    sp = int(_os.environ.get("ATTN_LSPL"))
                    nc.scalar.activation(
                        pt[:, 0:sp],
                        st[:, 0:sp],
                        mybir.ActivationFunctionType.Exp,
                        bias=bias_t[:],
                        scale=4.0,
                    )
                    nc.vector.tensor_scalar(
                        pt[:, sp:1024].bitcast(I16),
                        st[:, sp:1024],
                        SH_A,
                        0.0,
                        mybir.AluOpType.mult,
                        mybir.AluOpType.max,
                    )
                elif u == len(units) - 1 or eng[u] == "S":
                    nc.scalar.activation(
                        pt[:],
                        st[:],
                        mybir.ActivationFunctionType.Exp,
                        bias=bias_t[:],
                        scale=4.0,
                    )
                else:
                    nc.vector.tensor_scalar(
                        pt[:].bitcast(I16),
                        st[:],
                        SH_A,
                        0.0,
                        mybir.AluOpType.mult,
                        mybir.AluOpType.max,
                    )
                pts[u] = pt

            def emit_av(u):
                b, kt, t, half, first, last = units[u]
                if first and half == 0:
                    ots[b] = otpool.tile(
                        [128, 512], F32, tag="ot", name=f"ot{b}"
                    )
                    nc.tensor.matmul(
                        ots[b][:, 0:16 * VO_W],
                        vo_t[:, 0:128],
                        zvo_t[:],
                        start=True,
                        stop=False,
                        skip_group_check=True,
                    )
                ot = ots[b]
                pt = pts.pop(u)
                for qt in range(8):
                    qg = half * 8 + qt
                    nc.tensor.matmul(
                        ot[:, qg * VO_W:(qg + 1) * VO_W],
                        pt[:, qt * 128:(qt + 1) * 128],
                        vo_t[:, t * VO_W:(t + 1) * VO_W],
                        start=False,
                        stop=last,
                        skip_group_check=True,
                    )
                hw_ = 8 * VO_W
                if last and half == 0:
                    # the final batch's first-half regions are complete 14
                    # units before its last unit: copy+DMA them now so the
                    # drain tail only handles regions 8-15
                    obs[b] = obpool.tile(
                        [128, 16 * VO_W], F32, tag="ob", name=f"ob{b}"
                    )
                    if b % 2 == 0:
                        nc.scalar.copy(obs[b][:, 0:hw_], ot[:, 0:hw_])
                    else:
                        nc.vector.tensor_copy(
                            obs[b][:, 0:hw_], ot[:, 0:hw_]
                        )
                    nc.sync.dma_start(out_d[b][:, 0:hw_], obs[b][:, 0:hw_])
                if last and half == 1:
                    ob = obs[b]
                    if b == B - 1:
                        qw = 12 * VO_W
                        nc.scalar.copy(ob[:, hw_:qw], ot[:, hw_:qw])
                        nc.vector.tensor_copy(
                            ob[:, qw:16 * VO_W], ot[:, qw:16 * VO_W]
                        )
                    elif copy_eng[b] == "S":
                        nc.scalar.copy(
                            ob[:, hw_:16 * VO_W], ot[:, hw_:16 * VO_W]
                        )
                    else:
                        nc.vector.tensor_copy(
                            ob[:, hw_:16 * VO_W], ot[:, hw_:16 * VO_W]
                        )
                    nc.sync.dma_start(
                        out_d[b][:, hw_:16 * VO_W], ob[:, hw_:16 * VO_W]
                    )

            import os

            lag = int(os.environ.get("ATTN_LAG", "6"))
            for u in range(len(units)):
                emit_st(u)
                if u >= lag:
                    emit_av(u - lag)
            for u in range(len(units) - lag, len(units)):
                emit_av(u)

    nc.compile()
    return nc


def _f8parts(x):
    """fp8 e4m3 hi/mid/lo split: x ~ hi + m16/16 + l256/256."""
    hi = x.astype(f8).astype(np.float32)
    r1 = x - hi
    m16 = (r1 * np.float32(16.0)).astype(f8).astype(np.float32)
    r2 = r1 - m16 / np.float32(16.0)
    l256 = (r2 * np.float32(256.0)).astype(f8)
    return hi, m16.astype(f8), l256


def _term_rows(hi, m16, l256):
    """The six stored fp8 row-arrays for one operand side, scale-balanced
    so products reconstruct hi*hi + hi*m + m*hi + m*m + hi*l + l*hi.
    Order: [hh, hm, mh, mm, hl, lh]; q-side uses index 0..5 as-is, k-side
    pairs with the same index."""
    hi32 = hi.astype(np.float32)
    m32 = m16.astype(np.float32)
    l32 = l256.astype(np.float32)
    return [
        hi,                                     # hh
        (hi32 * 2.0 ** -3).astype(f8),          # hm (q side) / mh (k side)
        (m32 * 2.0 ** -1).astype(f8),           # mh (q side) / hm (k side)
        (m32 * 2.0 ** -4).astype(f8),           # mm
        (hi32 * 2.0 ** -5).astype(f8),          # hl / lh
        (l32 * 2.0 ** -3).astype(f8),           # lh / hl
    ]


def kernel(key_and_value, query, seq_len):
    key_and_value = np.asarray(key_and_value, dtype=np.float32)
    query = np.asarray(query, dtype=np.float32)
    sl = np.asarray(seq_len).reshape(-1).astype(np.int64)

    nbs = tuple(int(-(-int(s) // KT_TILE)) for s in sl)
    nb_total = sum(nbs)

    if nbs not in _cache:
        _cache[nbs] = _build(nbs)
    nc = _cache[nbs]

    k_all = key_and_value[:, :, :128].copy()  # [B, S, 128]
    v_all = key_and_value[:, :, 128:].copy()

    # zero invalid key/value rows (replaces the mask bias)
    valids = []
    for b in range(B):
        nrow = nbs[b] * 128
        valid = (np.arange(nrow) < sl[b]).astype(np.float32)
        valids.append(valid)
        k_all[b, int(sl[b]):nrow] = 0.0
        v_all[b, int(sl[b]):nrow] = 0.0

    q_all_t = query.transpose(0, 2, 1)  # [B, 128, S]
    q_rows_a = _term_rows(*_f8parts(q_all_t))  # each [B, 128, S] f8
    k_rows_a = _term_rows(*_f8parts(k_all))  # each [B, S, 128] f8
    K_PERM = [0, 2, 1, 3, 5, 4]  # k-side row-array index per term

    in_maps = []
    for h in range(H):
        c0 = h * DH
        qT = np.empty((B, 98, S), dtype=f8)
        for i in range(6):
            qT[:, i * DH:(i + 1) * DH] = q_rows_a[i][:, c0:c0 + DH]
        qT[:, 96] = f8(1.0)
        qT[:, 97] = f8(2.0 ** -6)
        # DoubleRow layout: row c -> [c//2, (c%2)*S + col]
        qT_dr = np.ascontiguousarray(qT.reshape(B, 49, 2 * S))
        kT_chunks = []
        vo_chunks = []
        for b in range(B):
            nrow = nbs[b] * 128
            kc = np.empty((98, nrow), dtype=f8)
            for i in range(6):
                kc[i * DH:(i + 1) * DH] = (
                    k_rows_a[K_PERM[i]][b, :nrow, c0:c0 + DH].T
                )
            kc[96] = (np.float32(DHI) * valids[b][:nrow]).astype(f8)
            kc[97] = (
                np.float32(DLO * 64.0) * valids[b][:nrow]
            ).astype(f8)
            # [98, nrow] -> [49, 2, nb, 128] -> [49, nb*256]
            kT_chunks.append(
                kc.reshape(49, 2, nbs[b], 128)
                .transpose(0, 2, 1, 3)
                .reshape(49, nbs[b] * 256)
            )
            vb = v_all[b, :nrow, c0:c0 + DH].reshape(nbs[b], 128, DH)
            vo_b = np.empty((nbs[b], 128, VO_W), dtype=bf16)
            vo_b[:, :, :DH] = vb.astype(bf16)
            vo_b[:, :, DH] = valids[b].reshape(nbs[b], 128).astype(bf16)
            vo_chunks.append(
                vo_b.transpose(1, 0, 2).reshape(128, nbs[b] * VO_W)
            )
        kT = np.ascontiguousarray(np.concatenate(kT_chunks, axis=1))
        vo = np.ascontiguousarray(np.concatenate(vo_chunks, axis=1))
        in_maps.append({
            "qT": qT_dr,
            "kT": kT,
            "vo": vo,
        })

    import os

    trace = bool(os.environ.get("ATTN_TRACE"))
    kw = {}
    if trace:
        kw = dict(
            trace=True,
            tmpdir=os.environ.get("ATTN_TRACE_DIR") or None,
            trace_cores=[0],
        )
    res = run_bass_kernel_spmd(nc, in_maps, core_ids=list(range(H)), **kw)
    if trace and res.exec_time_ns is not None:
        print(f"HW exec time: {res.exec_time_ns} ns")
        kernel.last_exec_time_ns = res.exec_time_ns

    out = np.empty((B, S, H * DH), dtype=np.float32)
    for h in range(H):
        o = res.results[h]["outT"].reshape(B, 128, 16, VO_W)
        num = o[:, :, :, :DH]  # [B, 128qp, 16qt, 16]
        den = o[:, :, :, DH]  # [B, 128qp, 16qt]
        val = num / den[:, :, :, None]
        # q position = qt*128 + qp
        out[:, :, h * DH:(h + 1) * DH] = val.transpose(0, 2, 1, 3).reshape(
            B, S, DH
        )
    return out


# revision 76
# speedup vs baseline: 1.0081x; 1.0081x over previous
"""Multi-head attention (B=4, S=2048, H=8 heads, d_head=16) on 8 trn2 cores.

Sharding: one head per core. Per head/batch, masked-softmax attention with a
transposed-scores dataflow and a two-engine exp pipeline:

    S^T[k, q] = matmul(lhsT=K_dT[49,2,128], rhs=Q_dT[49,2,512])  fp8 e4m3
        DoubleRow (0.5 cycles/row): Q and K split hi/mid/lo in fp8 on the
        host with scale-balanced pairings (6 cross terms, ~0.003 logit
        err), plus two shift rows (3.25 + dlo) * valid(k) that pre-bias
        the logits so both exp paths below need no additive constant.
        Invalid key columns are zeroed on the host (replaces the -1e30
        mask: their weights underflow to 0).

    P^T = exp(4*S^T - 75)  computed on TWO engines, alternating per unit
        (unit = one k-tile x 1024 q columns; greedy weighted assignment,
        last unit forced to ScalarE to shorten the drain critical path):
        - ScalarE: activation Exp (exact), bias = -75 - 4*(3.25+dlo)
        - DVE: Schraudolph bit-trick exp: int16 bits = rint(max(st*A, 0))
          with A = 4*128/ln2, written through an int16 bitcast view of the
          bf16 P^T tile. Round-to-nearest + saturation verified on HW.
          ~+-3% relative error on ~half the weights; rel-err budget 2e-2.

    out^T[q, j] += matmul(lhsT=P^T[128,128q], rhs=VO[128,17])  per q-tile:
        transposed AV: P^T is the stationary operand, the moving operand is
        the tiny [128 keys, 17] V|ones block, so each AV matmul streams only
        17 rows. Column 16 (ones*valid) accumulates the softmax denominator.
        Accumulated over k-tiles in PSUM; host divides num/den.

Weight loads are free in the cost model; matmul cost = moving free size.
Engine-busy per unit: ScalarE ~1038ns, DVE ~1192ns, PE ~270ns. Greedy
weighted assignment keeps both exp engines saturated; the kernel is
exp-throughput-bound at ~0.58us/unit (76 units). AV matmuls are emitted
lag-6 behind S^T so the in-order PE never blocks st production on an
in-flight exp; PSUM: 3 st bufs (6 banks) + 2 ot accumulators (2 banks).
A later start=True matmul in a PSUM bank discards other regions'
uncommitted accumulation (verified on HW), so each batch's ot bank is
opened by one zeroing start=True matmul and all AV matmuls accumulate
with start=False.

DMA: the transfer server is one serialized FIFO (by generation-end +
DGE delay) and HWDGE generation is a single engine shared by the
sync/scalar rings, so the critical first-unit set is kept to TWO
generations running in parallel (a strided 3D-AP DMA fetches both
DoubleRow r-slices of the q half in one go on sync HWDGE; kT b0 rides
first-in-queue on the Pool SWDGE path) while everything else queues
behind. Output copies split at the q-half boundary: under half-major
unit order a batch's first 8 regions finish many units before its
last, so their PSUM->SBUF copy + DMA hide entirely in steady state.
"""

import ml_dtypes
import numpy as np

import concourse.bass as bass
import concourse.tile as tile
from concourse import bacc, mybir
from concourse.ap import AP as BassAP
from concourse.bass_utils import run_bass_kernel_spmd


def _ap3(ap, d1, d1_stride):
    """Expand a 2D AP [P, N] to [P, d1, N] with the middle dim strided —
    the [partition, 2, moving] layout DoubleRow matmuls expect."""
    (pstride, pcount), (fstride, fcount) = list(ap.ap)
    assert fstride == 1
    return BassAP(
        ap.tensor, ap.offset,
        [[pstride, pcount], [d1_stride, d1], [1, fcount]],
    )

B = 4
S = 2048
H = 8
DH = 16
KT_TILE = 128
VO_W = 17  # 16 v dims + ones column

bf16 = ml_dtypes.bfloat16
f8 = ml_dtypes.float8_e4m3

DHI = 3.25
# shift-lo row is stored in fp8 as f8(64*dlo) paired with q-row 2^-6
DLO = float(np.float32(np.asarray(-7.019043e-04 * 64.0, dtype=np.float32).astype(f8)) / 64.0)
SH_A = 738.65986  # 4 * 128 / ln(2), f32
BIAS = -75.0 - 4.0 * (DHI + DLO)  # ScalarE activation bias

F32 = mybir.dt.float32
BF16 = mybir.dt.bfloat16
I16 = mybir.dt.int16
F8E4 = mybir.dt.float8e4

# per-unit engine busy estimates (ns) for the greedy assignment
import os as _os
SC_UNIT = 1024 * 0.8333 + 185
DV_ADJ = float(_os.environ.get("ATTN_DADJ", "40"))
DV_UNIT = 1024 * 1.0417 + 125
SC_COPY = 272 * 0.8333 + 185
DV_COPY = 272 * 1.0417 + 125

_cache = {}


def _build(nbs):
    nb_total = sum(nbs)

    nc = bacc.Bacc(
        "TRN2",
        target_bir_lowering=False,
        debug=False,
        num_devices=8,
    )

    # fp8 DoubleRow layouts: 98 contraction rows as [49 partitions, 2]
    # qT: [49, r*2048 + q]; kT: [49, t*256 + r*128 + m]
    qT_d = nc.dram_tensor(
        "qT", [B, 49, 2 * S], F8E4, kind="ExternalInput"
    ).ap()
    kT_d = nc.dram_tensor(
        "kT", [49, nb_total * 256], F8E4, kind="ExternalInput"
    ).ap()
    vo_d = nc.dram_tensor(
        "vo", [128, nb_total * VO_W], BF16, kind="ExternalInput"
    ).ap()
    out_d = nc.dram_tensor(
        "outT", [B, 128, 16 * VO_W], F32, kind="ExternalOutput"
    ).ap()

    with tile.TileContext(nc) as tc:
        with (
            tc.tile_pool(name="const", bufs=1) as const,
            tc.tile_pool(name="pt", bufs=9) as ptpool,
            tc.tile_pool(name="st", bufs=3, space="PSUM") as stpool,
            tc.tile_pool(name="ot", bufs=2, space="PSUM") as otpool,
            tc.tile_pool(name="ob", bufs=2) as obpool,
        ):
            q_tiles = [
                const.tile([49, 2 * S], F8E4, tag=f"qT{b}", name=f"qT{b}")
                for b in range(B)
            ]
            kT_t = const.tile([49, nb_total * 256], F8E4, tag="kT")
            vo_t = const.tile([128, nb_total * VO_W], BF16, tag="vo")

            # Critical-path DMAs first: kT tile 0 + vo on the sync HWDGE
            # ring, qT batch 0 in 512-col chunks on the scalar ring so the
            # first S^T matmul's input lands as early as possible. Bulk qT
            # for later batches rides the sync ring; bulk kT via gpsimd
            # SWDGE (Pool engine is otherwise idle).
            # DMA plan. The DMA transfer server is a single serialized
            # resource (FIFO by eligibility), and HWDGE generation is one
            # engine shared by the sync/scalar rings (~630ns per DMA) while
            # SWDGE generation runs separately on the Pool engine. Critical
            # order: qc0, qc1, kT0 feed the first unit; qc2+qc3 (second
            # half of qT b0) go via SWDGE so their generation overlaps;
            # bulk rides the sync ring BEHIND the critical set so its
            # transfers queue after them.
            pewarm = const.tile([98, 512], BF16, tag="pewarm")
            nc.vector.memset(pewarm[:], 0.0)
            # critical (HWDGE, 3 generations): both r-slices of qT b0's
            # first q-half on the scalar ring + all of kT b0 on sync.
            # The h1 q-slices and vo ride SWDGE (Pool) so they don't take
            # HWDGE slots ahead of the critical set.
            # one strided DMA brings BOTH r-slices of a q half: critical
            # set is just two generations (q-h0 on sync HWDGE, kT b0 first
            # on SWDGE), running in parallel
            nc.gpsimd.dma_start(
                kT_t[:, 0:nbs[0] * 256], kT_d[:, 0:nbs[0] * 256]
            )
            nc.sync.dma_start(
                _ap3(q_tiles[0][:, 0:1024], 2, 2048),
                _ap3(qT_d[0][:, 0:1024], 2, 2048),
            )
            nc.gpsimd.dma_start(
                _ap3(q_tiles[0][:, 1024:2048], 2, 2048),
                _ap3(qT_d[0][:, 1024:2048], 2, 2048),
            )
            nc.sync.dma_start(vo_t[:], vo_d)
            # Bias column for the ScalarE Exp activation.
            bias_t = const.tile([128, 1], F32, tag="bias")
            nc.vector.memset(bias_t[:], BIAS)
            # Zero moving-operand for the per-batch PSUM-bank-opening
            # matmul (see emit_av: one start=True matmul zeroes the ot
            # region; all real AV matmuls then accumulate with
            # start=False, since a later start=True in the same bank
            # discards other regions' uncommitted accumulation).
            zvo_t = const.tile([128, 16 * VO_W], BF16, tag="zvo")
            nc.vector.memset(zvo_t[:], 0.0)
            # Prefetch the exp table set on ScalarE while input DMAs run.
            warm = const.tile([1, 1], F32, tag="warm")
            nc.vector.memset(warm[:], 0.0)
            nc.scalar.activation(
                warm[:], warm[:], mybir.ActivationFunctionType.Exp
            )
            # Two dummy matmuls start the PE p-state ramp clock early; the
            # ramp counts wall time from the first PE instruction, so the
            # steady-state matmuls all run at full clock.
            st_w = stpool.tile([128, 1024], F32, tag="st")
            for j in range(2):
                nc.tensor.matmul(
                    st_w[:, 512 * j:512 * (j + 1)],
                    pewarm[:, 0:128],
                    pewarm[:],
                    start=True,
                    stop=True,
                )
            for b in range(1, B):
                off = sum(nbs[:b])
                nb = nbs[b]
                ring = nc.sync if b == 1 else nc.gpsimd
                ring.dma_start(q_tiles[b][:], qT_d[b])
                ring.dma_start(
                    kT_t[:, off * 256:(off + nb) * 256],
                    kT_d[:, off * 256:(off + nb) * 256],
                )

            # Unit list: one unit = one k-tile x 1024 q columns. Batch 0 is
            # half-major (all half-0 units first) so the first units only
            # need the first half of qT batch 0 — its 512-col DMA chunks
            # arrive serially on the scalar ring.
            units = []
            for b in range(B):
                if b == 0:
                    for half in range(2):
                        for kt in range(nbs[0]):
                            units.append(
                                (b, kt, kt, half, kt == 0, kt == nbs[0] - 1)
                            )
                else:
                    for half in range(2):
                        for kt in range(nbs[b]):
                            t = sum(nbs[:b]) + kt
                            units.append(
                                (b, kt, t, half, kt == 0, kt == nbs[b] - 1)
                            )

            # Greedy weighted engine assignment for the exp units (+ the
            # batch-end PSUM->SBUF copies).
            eng = []
            busy = {"S": 0.0, "D": 0.0}
            copy_eng = {}
            for u, (b, kt, t, half, first, last) in enumerate(units):
                if busy["S"] + SC_UNIT <= busy["D"] + DV_UNIT + DV_ADJ:
                    eng.append("S")
                    busy["S"] += SC_UNIT
                else:
                    eng.append("D")
                    busy["D"] += DV_UNIT
                if last and half == 1:
                    if busy["S"] + SC_COPY <= busy["D"] + DV_COPY:
                        copy_eng[b] = "S"
                        busy["S"] += SC_COPY
                    else:
                        copy_eng[b] = "D"
                        busy["D"] += DV_COPY

            sts = {}
            pts = {}
            ots = {}
            obs = {}

            def emit_st(u):
                b, kt, t, half, first, last = units[u]
                st = stpool.tile([128, 1024], F32, tag="st")
                for j in range(2):
                    qs = 1024 * half + 512 * j
                    nc.tensor.matmul(
                        st[:, 512 * j:512 * (j + 1)],
                        _ap3(kT_t[:, t * 256:(t + 1) * 256][:, 0:128], 2, 128),
                        _ap3(q_tiles[b][:, qs:qs + 512], 2, 2048),
                        start=True,
                        stop=True,
                        perf_mode=mybir.MatmulPerfMode.DoubleRow,
                    )
                pt = ptpool.tile([128, 1024], BF16, tag="pt")
                if u == len(units) - 1 and _os.environ.get("ATTN_LSPL"):
                    sp = int(_os.environ.get("ATTN_LSPL"))
                    nc.scalar.activation(
                        pt[:, 0:sp],
                        st[:, 0:sp],
                        mybir.ActivationFunctionType.Exp,
                        bias=bias_t[:],
                        scale=4.0,
                    )
                    nc.vector.tensor_scalar(
                        pt[:, sp:1024].bitcast(I16),
                        st[:, sp:1024],
                        SH_A,
                        0.0,
                        mybir.AluOpType.mult,
                        mybir.AluOpType.max,
                    )
                elif u == len(units) - 1 or eng[u] == "S":
                    nc.scalar.activation(
                        pt[:],
                        st[:],
                        mybir.ActivationFunctionType.Exp,
                        bias=bias_t[:],
                        scale=4.0,
                    )
                else:
                    nc.vector.tensor_scalar(
                        pt[:].bitcast(I16),
                        st[:],
                        SH_A,
                        0.0,
                        mybir.AluOpType.mult,
                        mybir.AluOpType.max,
                    )
                pts[u] = pt

            def emit_av(u):
                b, kt, t, half, first, last = units[u]
                if first and half == 0:
                    ots[b] = otpool.tile(
                        [128, 512], F32, tag="ot", name=f"ot{b}"
                    )
                    nc.tensor.matmul(
                        ots[b][:, 0:16 * VO_W],
                        vo_t[:, 0:128],
                        zvo_t[:],
                        start=True,
                        stop=False,
                        skip_group_check=True,
                    )
                ot = ots[b]
                pt = pts.pop(u)
                for qt in range(8):
                    qg = half * 8 + qt
                    nc.tensor.matmul(
                        ot[:, qg * VO_W:(qg + 1) * VO_W],
                        pt[:, qt * 128:(qt + 1) * 128],
                        vo_t[:, t * VO_W:(t + 1) * VO_W],
                        start=False,
                        stop=last,
                        skip_group_check=True,
                    )
                hw_ = 8 * VO_W
                if last and half == 0:
                    # the final batch's first-half regions are complete 14
                    # units before its last unit: copy+DMA them now so the
                    # drain tail only handles regions 8-15
                    obs[b] = obpool.tile(
                        [128, 16 * VO_W], F32, tag="ob", name=f"ob{b}"
                    )
                    if b % 2 == 0:
                        nc.scalar.copy(obs[b][:, 0:hw_], ot[:, 0:hw_])
                    else:
                        nc.vector.tensor_copy(
                            obs[b][:, 0:hw_], ot[:, 0:hw_]
                        )
                    nc.sync.dma_start(out_d[b][:, 0:hw_], obs[b][:, 0:hw_])
                if last and half == 1:
                    ob = obs[b]
                    if b != B - 1 and copy_eng[b] == "S":
                        nc.scalar.copy(
                            ob[:, hw_:16 * VO_W], ot[:, hw_:16 * VO_W]
                        )
                    else:
                        nc.vector.tensor_copy(
                            ob[:, hw_:16 * VO_W], ot[:, hw_:16 * VO_W]
                        )
                    nc.sync.dma_start(
                        out_d[b][:, hw_:16 * VO_W], ob[:, hw_:16 * VO_W]
                    )

            import os

            lag = int(os.environ.get("ATTN_LAG", "6"))
            for u in range(len(units)):
                emit_st(u)
                if u >= lag:
                    emit_av(u - lag)
            for u in range(len(units) - lag, len(units)):
                emit_av(u)

    nc.compile()
    return nc


def _f8parts(x):
    """fp8 e4m3 hi/mid/lo split: x ~ hi + m16/16 + l256/256."""
    hi = x.astype(f8).astype(np.float32)
    r1 = x - hi
    m16 = (r1 * np.float32(16.0)).astype(f8).astype(np.float32)
    r2 = r1 - m16 / np.float32(16.0)
    l256 = (r2 * np.float32(256.0)).astype(f8)
    return hi, m16.astype(f8), l256


def _term_rows(hi, m16, l256):
    """The six stored fp8 row-arrays for one operand side, scale-balanced
    so products reconstruct hi*hi + hi*m + m*hi + m*m + hi*l + l*hi.
    Order: [hh, hm, mh, mm, hl, lh]; q-side uses index 0..5 as-is, k-side
    pairs with the same index."""
    hi32 = hi.astype(np.float32)
    m32 = m16.astype(np.float32)
    l32 = l256.astype(np.float32)
    return [
        hi,                                     # hh
        (hi32 * 2.0 ** -3).astype(f8),          # hm (q side) / mh (k side)
        (m32 * 2.0 ** -1).astype(f8),           # mh (q side) / hm (k side)
        (m32 * 2.0 ** -4).astype(f8),           # mm
        (hi32 * 2.0 ** -5).astype(f8),          # hl / lh
        (l32 * 2.0 ** -3).astype(f8),           # lh / hl
    ]


def kernel(key_and_value, query, seq_len):
    key_and_value = np.asarray(key_and_value, dtype=np.float32)
    query = np.asarray(query, dtype=np.float32)
    sl = np.asarray(seq_len).reshape(-1).astype(np.int64)

    nbs = tuple(int(-(-int(s) // KT_TILE)) for s in sl)
    nb_total = sum(nbs)

    if nbs not in _cache:
        _cache[nbs] = _build(nbs)
    nc = _cache[nbs]

    k_all = key_and_value[:, :, :128].copy()  # [B, S, 128]
    v_all = key_and_value[:, :, 128:].copy()

    # zero invalid key/value rows (replaces the mask bias)
    valids = []
    for b in range(B):
        nrow = nbs[b] * 128
        valid = (np.arange(nrow) < sl[b]).astype(np.float32)
        valids.append(valid)
        k_all[b, int(sl[b]):nrow] = 0.0
        v_all[b, int(sl[b]):nrow] = 0.0

    q_all_t = query.transpose(0, 2, 1)  # [B, 128, S]
    q_rows_a = _term_rows(*_f8parts(q_all_t))  # each [B, 128, S] f8
    k_rows_a = _term_rows(*_f8parts(k_all))  # each [B, S, 128] f8
    K_PERM = [0, 2, 1, 3, 5, 4]  # k-side row-array index per term

    in_maps = []
    for h in range(H):
        c0 = h * DH
        qT = np.empty((B, 98, S), dtype=f8)
        for i in range(6):
            qT[:, i * DH:(i + 1) * DH] = q_rows_a[i][:, c0:c0 + DH]
        qT[:, 96] = f8(1.0)
        qT[:, 97] = f8(2.0 ** -6)
        # DoubleRow layout: row c -> [c//2, (c%2)*S + col]
        qT_dr = np.ascontiguousarray(qT.reshape(B, 49, 2 * S))
        kT_chunks = []
        vo_chunks = []
        for b in range(B):
            nrow = nbs[b] * 128
            kc = np.empty((98, nrow), dtype=f8)
            for i in range(6):
                kc[i * DH:(i + 1) * DH] = (
                    k_rows_a[K_PERM[i]][b, :nrow, c0:c0 + DH].T
                )
            kc[96] = (np.float32(DHI) * valids[b][:nrow]).astype(f8)
            kc[97] = (
                np.float32(DLO * 64.0) * valids[b][:nrow]
            ).astype(f8)
            # [98, nrow] -> [49, 2, nb, 128] -> [49, nb*256]
            kT_chunks.append(
                kc.reshape(49, 2, nbs[b], 128)
                .transpose(0, 2, 1, 3)
                .reshape(49, nbs[b] * 256)
            )
            vb = v_all[b, :nrow, c0:c0 + DH].reshape(nbs[b], 128, DH)
            vo_b = np.empty((nbs[b], 128, VO_W), dtype=bf16)
            vo_b[:, :, :DH] = vb.astype(bf16)
            vo_b[:, :, DH] = valids[b].reshape(nbs[b], 128).astype(bf16)
            vo_chunks.append(
                vo_b.transpose(1, 0, 2).reshape(128, nbs[b] * VO_W)
            )
        kT = np.ascontiguousarray(np.concatenate(kT_chunks, axis=1))
        vo = np.ascontiguousarray(np.concatenate(vo_chunks, axis=1))
        in_maps.append({
            "qT": qT_dr,
            "kT": kT,
            "vo": vo,
        })

    import os

    trace = bool(os.environ.get("ATTN_TRACE"))
    kw = {}
    if trace:
        kw = dict(
            trace=True,
            tmpdir=os.environ.get("ATTN_TRACE_DIR") or None,
            trace_cores=[0],
        )
    res = run_bass_kernel_spmd(nc, in_maps, core_ids=list(range(H)), **kw)
    if trace and res.exec_time_ns is not None:
        print(f"HW exec time: {res.exec_time_ns} ns")
        kernel.last_exec_time_ns = res.exec_time_ns

    out = np.empty((B, S, H * DH), dtype=np.float32)
    for h in range(H):
        o = res.results[h]["outT"].reshape(B, 128, 16, VO_W)
        num = o[:, :, :, :DH]  # [B, 128qp, 16qt, 16]
        den = o[:, :, :, DH]  # [B, 128qp, 16qt]
        val = num / den[:, :, :, None]
        # q position = qt*128 + qp
        out[:, :, h * DH:(h + 1) * DH] = val.transpose(0, 2, 1, 3).reshape(
            B, S, DH
        )
    return out
